# revision 35
# baseline (speedup 1.0000x reference)
"""DampedLinOSSLayer Trainium2 kernel (8 NeuronCores, batch-sharded).

Math: per SSM channel p, the complex diagonal recurrence
    x_t = lam_p * x_{t-1} + bu_t,   lam_p = r_p * exp(i*th_p)
is factored through the gauge x_t = exp(i*th_p*t) * y_t:
    y_t = r_p * y_{t-1} + c_t,      c_t = exp(-i*th_p*t) * bu_t
so the recurrence has a REAL per-channel coefficient and runs as the
hardware tensor_tensor_scan on DVE over the full L=2048 at once.

Rotation work runs in bf16 (DVE 2x mode) with a single shared
[cos|sin] table; the sign structure is absorbed into an add/sub pair
on the input side and into 4-term C-projection weights (PE) on the
output side:
    pre:  prod[comp,tab] = bu[comp] * E[tab]   (E = [cos|sin], DVE)
          c_re = prod[0,0] + prod[1,1]         (DVE add)
          c_im = prod[1,0] - prod[0,1]         (DVE sub)
    scan: y = r*y + c  along t (DVE, fp32 state, full 2048)
    post: q[comp,tab] = y[comp] * E[tab]       (Pool)
          out = +Cre.q00 - Cim.q01 - Cim.q10 - Cre.q11 + D.u  (PE)

Engine assignment is stall-aware: engines execute their queues in
order, so the entire pre->scan chain stays on DVE (no cross-engine
bubble), while the post-products -- consumed only by PE -- run on the
otherwise-idle Pool engine. Measured marginal HW time ~187us/core
(baseline 334us) via reps-loop wall-clock differencing.

The [t,h]<->[h,t] transposes are done on the HOST (numpy): the device
receives inputs pre-transposed as bf16 [B, H, L] and returns [B, H, L]
f32, so no PE transposes / PSUM round-trips are needed on-device.
SSM channels sit on partitions (2 halves of 128), time on the free
dim. Each core takes 4 of the 32 batches.
"""

import numpy as np

BATCH, LENGTH, HIDDEN, P = 32, 2048, 128, 256
N_CORES = 8
BPC = BATCH // N_CORES          # batches per core
L = LENGTH
CH = 512                        # PSUM-bank chunk for matmuls
NCH = L // CH

_COMPILED = {}


def _build_program(reps=1, add_engine="pool", post_pool=0, lead=0,
                   chunk_post=True):
    """reps>1 wraps the per-core batch loop in a hardware loop (timing).
    post_pool: number of (batch,half) streams whose post-product runs on
    Pool instead of DVE (load balancing)."""
    import concourse.bacc as bacc
    import concourse.mybir as mybir
    from concourse.tile import TileContext

    f32 = mybir.dt.float32
    bf16 = mybir.dt.bfloat16

    nc = bacc.Bacc("TRN2", target_bir_lowering=False, debug=False,
                   num_devices=N_CORES)

    # ---- DRAM tensors (per-core) ----
    xin = nc.dram_tensor("xin", [BPC, HIDDEN, L], bf16,
                         kind="ExternalInput").ap()
    bw = nc.dram_tensor("bw", [HIDDEN, 2, 2, 128], bf16,
                        kind="ExternalInput").ap()
    cw = nc.dram_tensor("cw", [128, 2, 4, HIDDEN], bf16,
                        kind="ExternalInput").ap()
    etab = nc.dram_tensor("etab", [128, 2, 2, L], bf16,
                          kind="ExternalInput").ap()
    rcol = nc.dram_tensor("rcol", [128, 2], f32, kind="ExternalInput").ap()
    dwb = nc.dram_tensor("dwb", [HIDDEN, HIDDEN], bf16,
                         kind="ExternalInput").ap()
    out = nc.dram_tensor("out", [BPC, HIDDEN, L], f32,
                         kind="ExternalOutput").ap()

    with TileContext(nc) as tc:
        import contextlib

        @contextlib.contextmanager
        def body_loop():
            if reps == 1:
                yield
            else:
                with tc.For_i(0, reps, 1):
                    yield

        with (
            tc.tile_pool(name="const", bufs=1) as cpool,
            tc.tile_pool(name="intp", bufs=2) as intr_pool,
            tc.tile_pool(name="busb", bufs=3) as busb_pool,
            tc.tile_pool(name="prod", bufs=2) as prod_pool,
            tc.tile_pool(name="ccb", bufs=2) as cc_pool,
            tc.tile_pool(name="yb", bufs=2) as y_pool,
            tc.tile_pool(name="qb", bufs=2) as q_pool,
            tc.tile_pool(name="otb", bufs=2) as ot_pool,
            tc.tile_pool(name="psb", bufs=2, space="PSUM") as psb,
            tc.tile_pool(name="pso", bufs=3, space="PSUM") as pso,
        ):
            eng_add = nc.gpsimd if add_engine == "pool" else nc.vector

            # ---- constants to SBUF ----
            # Small consts go first; the big rotation table (2MB) and C
            # weights are deferred past stream 0's input DMA / B-proj
            # emission so they don't delay the pipeline fill (they are
            # first consumed ~12us in, well after their DMA completes).
            bw_t = cpool.tile([HIDDEN, 2, 2, 128], bf16, tag="bw")
            cw_t = cpool.tile([128, 2, 4, HIDDEN], bf16, tag="cw")
            etab_t = cpool.tile([128, 2, 2, L], bf16, tag="etab")
            rcol_t = cpool.tile([128, 2], f32, tag="rcol")
            dw_t = cpool.tile([HIDDEN, HIDDEN], bf16, tag="dw")
            for src, dst in [(bw, bw_t), (rcol, rcol_t), (dwb, dw_t)]:
                nc.sync.dma_start(dst[:], src[:])

            def emit_big_consts():
                nc.sync.dma_start(etab_t[:, 0], etab[:, 0])
                nc.sync.dma_start(etab_t[:, 1], etab[:, 1])
                nc.sync.dma_start(cw_t[:], cw[:])

            if reps != 1:
                emit_big_consts()

            ctx_loop = body_loop()
            ctx_loop.__enter__()

            # Software-pipelined emission over 8 streams s=(batch, half):
            # pre-products lead the scan stage by LEAD streams so the DVE
            # queue never stalls on the Pool adds (engines run in-order).
            LEAD = lead
            NS = 2 * BPC
            inT_t = {}
            cc_t = {}
            yy_t = {}
            q4_t = {}

            def stage_front(s):
                b, half = divmod(s, 2)
                if half == 0:
                    inT = intr_pool.tile([HIDDEN, L], bf16, tag="inT",
                                         name=f"inT{b}")
                    nc.sync.dma_start(inT[:], xin[b])
                    inT_t[b] = inT
                inT = inT_t[b]
                # ---- B-proj per chunk -> busb [p, comp, L] bf16 ----
                busb = busb_pool.tile([128, 2, L], bf16, tag="busb")
                for J in range(NCH):
                    tsl = slice(CH * J, CH * (J + 1))
                    bu = psb.tile([128, 2, CH], f32, tag="bu")
                    for comp in range(2):
                        nc.tensor.matmul(
                            bu[:, comp, :],
                            bw_t[:, half, comp],
                            inT[:, tsl],
                            start=True, stop=True)
                    nc.scalar.copy(busb[:, :, tsl], bu[:])

                # big consts ride behind stream 0's input DMA / B-proj
                # but must be emitted before their first consumer below
                if s == 0 and reps == 1:
                    emit_big_consts()

                # ---- pre-rotation products (DVE, bf16 2x) ----
                prod = prod_pool.tile([128, 2, 2, L], bf16, tag="prod")
                nc.vector.tensor_mul(
                    prod[:],
                    busb[:].unsqueeze(2).broadcast_to([128, 2, 2, L]),
                    etab_t[:, half].unsqueeze(1).broadcast_to(
                        [128, 2, 2, L]))
                # ---- c_re / c_im ----
                cc = cc_pool.tile([128, 2, L], bf16, tag="cc",
                                  name=f"cc{s}")
                eng_add.tensor_add(
                    cc[:, 0, :], prod[:, 0, 0, :], prod[:, 1, 1, :])
                eng_add.tensor_sub(
                    cc[:, 1, :], prod[:, 1, 0, :], prod[:, 0, 1, :])
                cc_t[s] = cc

            def stage_scan(s):
                b, half = divmod(s, 2)
                cc = cc_t.pop(s)
                yy = y_pool.tile([128, 2, L], bf16, tag="yy",
                                 name=f"yy{s}")
                for comp in range(2):
                    nc.vector.tensor_tensor_scan(
                        yy[:, comp, :],
                        rcol_t[:, half:half + 1].broadcast_to([128, L]),
                        cc[:, comp, :],
                        0.0,
                        op0=mybir.AluOpType.mult,
                        op1=mybir.AluOpType.add)
                # ---- post-rotation products, chunked so the C-proj can
                # pipeline per 512-column chunk instead of waiting for
                # the full-L product (shortens the drain tail) ----
                q4 = q_pool.tile([128, 2, 2, L], bf16, tag=f"q{half}",
                                 name=f"q{half}_{b}")
                eng_post = nc.gpsimd if s < post_pool else nc.vector
                ncp = NCH if chunk_post else 1
                wp = L // ncp
                for J in range(ncp):
                    tsl = slice(wp * J, wp * (J + 1))
                    eng_post.tensor_mul(
                        q4[:, :, :, tsl],
                        yy[:, :, tsl].unsqueeze(2).broadcast_to(
                            [128, 2, 2, wp]),
                        etab_t[:, half, :, tsl].unsqueeze(1).broadcast_to(
                            [128, 2, 2, wp]))
                q4_t[s] = q4

            def stage_out(b):
                inT = inT_t[b]
                qs = [q4_t.pop(2 * b), q4_t.pop(2 * b + 1)]
                for J in range(NCH):
                    tsl = slice(CH * J, CH * (J + 1))
                    outT = pso.tile([HIDDEN, CH], f32, tag="outT")
                    first = True
                    for half in range(2):
                        for term in range(4):
                            nc.tensor.matmul(
                                outT[:],
                                cw_t[:, half, term],
                                qs[half][:, term // 2, term % 2, tsl],
                                start=first, stop=False)
                            first = False
                    nc.tensor.matmul(
                        outT[:], dw_t[:], inT[:, tsl],
                        start=False, stop=True)
                    osb = ot_pool.tile([HIDDEN, CH], f32, tag="osb")
                    nc.scalar.copy(osb[:], outT[:])
                    nc.sync.dma_start(out[b][:, tsl], osb[:])

            for s in range(NS + LEAD + 1):
                if s < NS:
                    stage_front(s)
                t = s - LEAD
                if 0 <= t < NS:
                    stage_scan(t)
                # C-proj for batch B once both its halves' q4 are ready,
                # delayed one extra slot to keep PE fed with B-projs.
                u = s - LEAD - 1
                if u >= 0 and u % 2 == 1:
                    stage_out(u // 2)

            ctx_loop.__exit__(None, None, None)

    nc.compile()
    return nc


def _host_constants(A_diag, G_diag, steps, B, C, D):
    """Parameter projection + eigenvalues + rotation tables (f64 on host)."""
    import ml_dtypes
    bf = ml_dtypes.bfloat16

    A = A_diag.astype(np.float64)
    G = G_diag.astype(np.float64)
    st = steps.astype(np.float64)
    step = 1.0 / (1.0 + np.exp(-st))
    g = np.maximum(G, 0.0)
    denom = np.maximum(step * step, 1e-6)
    s = step * g
    base = np.sqrt(np.maximum(1.0 + s, 1e-6))
    a_low = (2.0 + s - 2.0 * base) / denom
    a_high = (2.0 + s + 2.0 * base) / denom
    a = a_low + np.maximum(A - a_low, 0.0) - np.maximum(A - a_high, 0.0)
    S = 1.0 / (1.0 + step * g)
    T = S + 1.0 - step * step * S * a
    imag = np.sqrt(np.maximum(S - 0.25 * T * T, 0.0))
    lam = 0.5 * T + 1j * imag                      # [P] complex128
    r = np.abs(lam)
    th = np.angle(lam)

    t = np.arange(L, dtype=np.float64)
    cos_m = np.cos(th[:, None] * t[None, :])       # [P, L]
    sin_m = np.sin(th[:, None] * t[None, :])

    etab = np.zeros((128, 2, 2, L), bf)
    for half in range(2):
        psl = slice(128 * half, 128 * (half + 1))
        etab[:, half, 0, :] = cos_m[psl].astype(bf)
        etab[:, half, 1, :] = sin_m[psl].astype(bf)

    Br = B[..., 0].astype(np.float64)              # [P, H]
    Bi = B[..., 1].astype(np.float64)
    Cre = C[..., 0].astype(np.float64)             # [H, P]
    Cim = C[..., 1].astype(np.float64)

    bw = np.zeros((HIDDEN, 2, 2, 128), bf)
    cw = np.zeros((128, 2, 4, HIDDEN), bf)
    for half in range(2):
        psl = slice(128 * half, 128 * (half + 1))
        bw[:, half, 0] = Br[psl].T.astype(bf)      # lhsT [h, p]
        bw[:, half, 1] = Bi[psl].T.astype(bf)
        # out = Cre.q00 - Cim.q01 - Cim.q10 - Cre.q11  (q[comp,tab])
        cw[:, half, 0] = Cre[:, psl].T.astype(bf)
        cw[:, half, 1] = (-Cim[:, psl].T).astype(bf)
        cw[:, half, 2] = (-Cim[:, psl].T).astype(bf)
        cw[:, half, 3] = (-Cre[:, psl].T).astype(bf)

    rcol = np.zeros((128, 2), np.float32)
    rcol[:, 0] = r[:128]
    rcol[:, 1] = r[128:]
    dwb = np.diag(D.astype(np.float64)).astype(bf)
    return dict(bw=bw, cw=cw, etab=etab, rcol=rcol, dwb=dwb)


def kernel(inputs, A_diag, G_diag, steps, B, C, D):
    import ml_dtypes
    from concourse import bass_utils

    inputs = np.asarray(inputs, np.float32)
    consts = _host_constants(np.asarray(A_diag), np.asarray(G_diag),
                             np.asarray(steps), np.asarray(B), np.asarray(C),
                             np.asarray(D))

    if "prog" not in _COMPILED:
        _COMPILED["prog"] = _build_program(add_engine="dve", post_pool=7)
    nc = _COMPILED["prog"]

    # host-side transpose [B, L, H] -> [B, H, L] and cast to bf16
    xh = np.ascontiguousarray(inputs.transpose(0, 2, 1)).astype(
        ml_dtypes.bfloat16)
    in_maps = []
    for core in range(N_CORES):
        m = dict(consts)
        m["xin"] = np.ascontiguousarray(xh[BPC * core: BPC * (core + 1)])
        in_maps.append(m)
    res = bass_utils.run_bass_kernel_spmd(nc, in_maps,
                                          core_ids=list(range(N_CORES)))
    out = np.concatenate([res.results[i]["out"] for i in range(N_CORES)],
                         axis=0)                   # [B, H, L]
    return np.ascontiguousarray(out.transpose(0, 2, 1)).astype(np.float32)
